# revision 15
# baseline (speedup 1.0000x reference)
"""Causal self-attention on 8 Trainium2 NeuronCores.

Problem: x[4,2048,1024], Wq/Wk/Wv/Wo[1024,1024], H=16 heads, dh=64.
    q,k,v = x@W{q,k,v}.T ; per-head causal softmax(q k^T/8) v ; out = y@Wo.T

Sharding (hybrid data+tensor parallel over 8 cores):
  core c -> (batch b = c//2, head-group hg = c%2 of 8 heads = 512 dims).
  Each core computes a partial output out_c[b] = y_hg @ Wo[:, hg].T ; the
  host sums the two partials per batch (the Wo all-reduce done on host).

Per-core kernel: one software-pipelined instruction stream.
  The attention inner loop (S^T matmul -> exp on ACT -> PV matmul) is
  ACT-bound per iteration, while the QKV/output projections are pure PE
  work with idle ACT.  So projection matmul "filler units" are interleaved
  INTO the attention kt-loop so the PE never waits for exp:
    prologue   : DMA (small first chunks in consumption order), QKV t0
    attn(q0)   : filler = QKV(t1)      attn(q1): filler = QKV(t2)
    attn(q2)   : filler = QKV(t3)      attn(q3): filler = out-proj(q0..q2)
    epilogue   : out-proj(q3) split in two q-chunks (see below)
  kt is stepped in pairs and the PV-pairs trail the S-pairs by a few
  kt-steps so neither the exp latency nor the PSUM drains gate the PE.

  Causal restriction: for a diagonal k-tile (tile-local index m), only
  q >= 128*m can see it, so S/exp/PV operate on the q-slice [128m:512] of
  the q-tile, and the 0/1 triangle mask multiply only touches the single
  128-wide q-block on the diagonal itself.

  Softmax normalization per (head-pair g): V carries a ones column so PV
  also accumulates the row-sum (row 64 of y_ps).  After the last PV the
  y banks drain to a [128,TQ] SBUF tile (both heads stacked on the
  partition axis) and the rowsums lane-shift to partitions 0/1.  Then --
  deferred one head-group so it never delays the next group's masks in
  the DVE queue -- one batched reciprocal, a bf16 cast, ONE selector
  matmul (lhsT = E[2,128] with E[0,0:64]=1, E[1,64:128]=1) broadcasts
  1/rowsum_h0 across partitions 0:64 and 1/rowsum_h1 across 64:128 in a
  single PE op, and ONE DVE multiply writes the normalized bf16 yT.

  Tail: y(qi=3) for q in [0,384) is final after the kt=14 PV (kt=15 only
  touches q>=384), so the last group's epilogue splits in an A-chunk
  [0:384) and B-chunk [384:512).  The final out-projection runs as 8
  384-wide chains (A) + 8 128-wide chains (B) whose PSUM drains go to the
  then-idle ACT engine, results staged in two SBUF tiles and written with
  3 batched DMAs -- so the post-attention tail is short and dense instead
  of 8 full-width chains + 8 serial DMA triggers.

Precision: all matmul operands bf16 (PSUM accum fp32); softmax recip
f32, broadcast via bf16 (adds ~0.2% rms, budget is 2e-2). exp needs no
max-subtraction: S/8 ~ N(0,1), exp safe in fp32.  fp8 was evaluated and
rejected: e4m3 rms quantization error ~2.5% propagates ~1:1 through any
single matmul stage into the output and would blow the 2e-2 budget.

Measured on hw: 407us (naive) -> 265us (v1) -> this version.
"""

import sys

import numpy as np

sys.path.insert(0, "/opt/trn_rl_repo")

import concourse.bass as bass  # noqa: F401
from concourse import bacc
import concourse.mybir as mybir
import concourse.tile as tile
from concourse.bass_utils import run_bass_kernel_spmd

B, T, D, H, DH = 4, 2048, 1024, 16, 64
NCORES = 8
HPC = 8                 # heads per core
JJ = HPC * DH           # 512: per-core qkv head dims
P = 128
TQ = 512                # attention q tile (free dim of S^T matmul)
TK = 128                # attention k tile (partition dim of S^T)
NDT = D // P            # 8 d-tiles (contraction for stage 1)
NJT = JJ // P           # 4 j-tiles (head-pair tiles)
NTT = T // TQ           # 4 t-tiles of 512
NKT = T // TK           # 16 k-tiles of 128
NOT_ = D // P           # 8 output row tiles (stage 3)
VW = 66                 # V row width: 64 dh + 1 ones + 1 pad
QA = 384                # tail A-chunk width (final after kt=nkt-2)
F32 = mybir.dt.float32
BF16 = mybir.dt.bfloat16
MUL = mybir.AluOpType.mult
EXP = mybir.ActivationFunctionType.Exp
COPY = mybir.ActivationFunctionType.Copy
INTERLEAVE = True   # dispense filler units inside the attention kt loop
RESTRICT = True     # causal q-column restriction on diagonal k-tiles
RESERVE = 4         # fillers held back to cover the tail epilogue chains


def build_program():
    nc = bacc.Bacc()
    # x / wq / wk are stored dt-INTERLEAVED in DRAM ([p, t, n] / [p, j, n]):
    # the first-consumed slices (a t-window x all dt; a j-tile x all dt) are
    # then contiguous per partition, so the startup DMAs move 2-8KB
    # descriptors instead of 256B-1KB ones (~4x the per-queue rate)
    xS = nc.dram_tensor("xS", [P, T, NDT], BF16, kind="ExternalInput")
    wqS = nc.dram_tensor("wqS", [P, JJ, NDT], BF16, kind="ExternalInput")
    wkS = nc.dram_tensor("wkS", [P, JJ, NDT], BF16, kind="ExternalInput")
    wvT = nc.dram_tensor("wvT", [D, JJ], BF16, kind="ExternalInput")
    woT = nc.dram_tensor("woT", [JJ, D], BF16, kind="ExternalInput")
    trid = nc.dram_tensor("tri", [P, P], BF16, kind="ExternalInput")
    maskd = nc.dram_tensor("mask", [4, P, TQ], BF16, kind="ExternalInput")
    outT = nc.dram_tensor("outT", [D, T], BF16, kind="ExternalOutput")

    outv = outT.rearrange("(n p) t -> n p t", p=P)     # [8,128,2048]
    outPv = outT.rearrange("(n p) t -> p n t", p=P)    # [128,8,2048]

    with tile.TileContext(nc) as tc:
        with (
            tc.tile_pool(name="persist", bufs=1) as persist,
            tc.tile_pool(name="ppool", bufs=2, space="PSUM") as ppool,
            tc.tile_pool(name="psS", bufs=2, space="PSUM") as psS,
            tc.tile_pool(name="psY", bufs=1, space="PSUM") as psY,
            tc.tile_pool(name="ptp", bufs=10) as ptp,
            tc.tile_pool(name="small", bufs=2) as small,
        ):
            # ---- persistent SBUF tensors ----
            x_sb = persist.tile([P, T, NDT], BF16)        # x^T, dt-interleaved
            wq_sb = persist.tile([P, JJ, NDT], BF16)
            wk_sb = persist.tile([P, JJ, NDT], BF16)
            wv_sb = persist.tile([P, NDT, JJ], BF16)
            wo_sb = persist.tile([P, NJT, D], BF16)
            qt_sb = persist.tile([P, NJT, T], BF16)       # QT [j,t]
            kt_sb = persist.tile([P, NJT, T], BF16)       # KT [j,t]
            v_sb = persist.tile([P, NKT, HPC, VW], BF16)  # V'[t, kt, h, dh|1]
            yt_sb = persist.tile([P, NJT, T], BF16)       # yT [i,t] normalized
            tri_sb = persist.tile([P, 1, P], BF16)        # causal 0/1 triangle
            ones_bf = persist.tile([1, DH], BF16)         # bc lhsT
            obA = persist.tile([P, NOT_, QA], BF16)       # tail A out staging
            obB = persist.tile([P, NOT_, TQ - QA], BF16)  # tail B out staging

            # ones column of V' (strided memset across kt,h); bc ones row
            nc.any.memset(v_sb[:, :, :, DH : DH + 1], 1.0)
            nc.any.memset(ones_bf[:], 1.0)

            mask_sb = persist.tile([P, 4, TQ], BF16)

            # ---- DMAs: small chunks first, in exact consumption order
            # (sync-queue trigger ~650ns each; transfers run concurrently
            # across queues, so the FIRST chunks must be small to cut the
            # first-matmul latency, the bulk stays coarse)
            wvP = wvT.rearrange("(n p) j -> p n j", p=P)
            woP = woT.rearrange("(n p) o -> p n o", p=P)    # [128,4,1024]
            nc.sync.dma_start(out=wq_sb[:, 0:P, :], in_=wqS[:, 0:P, :])
            nc.sync.dma_start(out=x_sb[:, 0:P, :], in_=xS[:, 0:P, :])
            nc.sync.dma_start(out=wk_sb[:, 0:P, :], in_=wkS[:, 0:P, :])
            nc.sync.dma_start(out=x_sb[:, P : 2 * P, :], in_=xS[:, P : 2 * P, :])
            nc.sync.dma_start(out=tri_sb[:, 0, :], in_=trid[:, :])
            nc.sync.dma_start(out=x_sb[:, 2 * P : 3 * P, :],
                              in_=xS[:, 2 * P : 3 * P, :])
            nc.sync.dma_start(out=x_sb[:, 3 * P : TQ, :],
                              in_=xS[:, 3 * P : TQ, :])
            nc.sync.dma_start(out=wq_sb[:, P:JJ, :], in_=wqS[:, P:JJ, :])
            nc.sync.dma_start(out=wk_sb[:, P:JJ, :], in_=wkS[:, P:JJ, :])
            nc.sync.dma_start(out=wv_sb[:, 0:2, :], in_=wvP[:, 0:2, :])
            nc.sync.dma_start(out=wv_sb[:, 2:4, :], in_=wvP[:, 2:4, :])
            nc.sync.dma_start(out=wv_sb[:, 4:6, :], in_=wvP[:, 4:6, :])
            nc.sync.dma_start(out=wv_sb[:, 6:8, :], in_=wvP[:, 6:8, :])
            if not RESTRICT:
                for m in range(4):
                    nc.sync.dma_start(out=mask_sb[:, m, :], in_=maskd[m])
            nc.sync.dma_start(out=x_sb[:, TQ : 2 * TQ, :],
                              in_=xS[:, TQ : 2 * TQ, :])
            nc.sync.dma_start(out=x_sb[:, 2 * TQ : T, :],
                              in_=xS[:, 2 * TQ : T, :])
            nc.sync.dma_start(out=wo_sb[:], in_=woP)

            # ---- filler units (each: one PSUM matmul chain + drain) ----
            def qk_unit(ti, w_sb, o_sb, jt, tw=TQ):
                # tw < TQ: t-chunked chains (prologue: each chunk gates on
                # its own x DMA so the PE starts before all of x-t0 lands)
                def run():
                    jsl = slice(jt * P, (jt + 1) * P)
                    ps = ppool.tile([P, TQ], F32, tag="mm",
                                    name=f"qk_{ti}_{jt}")
                    for tc in range(TQ // tw):
                        csl = slice(tc * tw, (tc + 1) * tw)
                        tsl = slice(ti * TQ + tc * tw, ti * TQ + (tc + 1) * tw)
                        for dt_ in range(NDT):
                            nc.tensor.matmul(
                                ps[:, csl],
                                lhsT=w_sb[:, jsl, dt_],
                                rhs=x_sb[:, tsl, dt_],
                                start=(dt_ == 0),
                                stop=(dt_ == NDT - 1),
                            )
                    tsl = slice(ti * TQ, (ti + 1) * TQ)
                    nc.vector.tensor_copy(o_sb[:, jt, tsl], ps[:])
                return run

            def v_unit(ti, tsub):
                def run():
                    kt_idx = ti * (TQ // P) + tsub
                    ssl = slice(ti * TQ + tsub * P, ti * TQ + (tsub + 1) * P)
                    ps = ppool.tile([P, JJ], F32, tag="mm",
                                    name=f"v_{kt_idx}")
                    for dt_ in range(NDT):
                        nc.tensor.matmul(
                            ps[:],
                            lhsT=x_sb[:, ssl, dt_],
                            rhs=wv_sb[:, dt_, :],
                            start=(dt_ == 0),
                            stop=(dt_ == NDT - 1),
                        )
                    nc.scalar.activation(
                        v_sb[:, kt_idx, :, 0:DH],
                        ps[:].rearrange("p (h i) -> p h i", h=HPC),
                        COPY,
                    )
                return run

            def o_unit(qi, ot):
                def run():
                    tsl = slice(qi * TQ, (qi + 1) * TQ)
                    osl = slice(ot * P, (ot + 1) * P)
                    ps = ppool.tile([P, TQ], F32, tag="mm",
                                    name=f"o_{qi}_{ot}")
                    for it in range(NJT):
                        nc.tensor.matmul(
                            ps[:],
                            lhsT=wo_sb[:, it, osl],
                            rhs=yt_sb[:, it, tsl],
                            start=(it == 0),
                            stop=(it == NJT - 1),
                        )
                    ob = small.tile([P, TQ], BF16, tag="ost", bufs=3,
                                    name=f"ob_{qi}_{ot}")
                    if fstate.get("drain") == "act":
                        # tail: ACT is done with exp; draining there keeps
                        # the PE's filler stream independent of the DVE
                        # queue (which is busy with the epilogue chains)
                        nc.scalar.activation(ob[:], ps[:], COPY)
                    else:
                        nc.vector.tensor_copy(ob[:], ps[:])
                    nc.sync.dma_start(out=outv[ot][:, tsl], in_=ob[:])
                return run

            # tail out-proj: q-chunked chains, ACT drains into staging tiles
            def o_tail_unit(ot, q0, q1, stage):
                def run():
                    tsl = slice(3 * TQ + q0, 3 * TQ + q1)
                    osl = slice(ot * P, (ot + 1) * P)
                    ps = ppool.tile([P, TQ], F32, tag="mm",
                                    name=f"oT_{ot}_{q0}")
                    for it in range(NJT):
                        nc.tensor.matmul(
                            ps[:, 0 : q1 - q0],
                            lhsT=wo_sb[:, it, osl],
                            rhs=yt_sb[:, it, tsl],
                            start=(it == 0),
                            stop=(it == NJT - 1),
                        )
                    nc.scalar.activation(
                        stage[:, ot, :], ps[:, 0 : q1 - q0], COPY
                    )
                return run

            def units_t(ti):
                return (
                    [qk_unit(ti, wq_sb, qt_sb, jt) for jt in range(NJT)]
                    + [qk_unit(ti, wk_sb, kt_sb, jt) for jt in range(NJT)]
                    + [v_unit(ti, ts) for ts in range(TQ // P)]
                )

            def units_o(qi):
                return [o_unit(qi, ot) for ot in range(NOT_)]

            # ---- prologue: only the two units attn(q0, g0) needs,
            # t-chunked so the PE starts on the first 128-col x chunk ----
            qk_unit(0, wq_sb, qt_sb, 0, tw=P)()
            qk_unit(0, wk_sb, kt_sb, 0, tw=P)()

            def units_t0_rest():
                # qk(g1) early for the g1 S-stream; V before the first pops
                us = [qk_unit(0, wq_sb, qt_sb, 1),
                      qk_unit(0, wk_sb, kt_sb, 1)]
                us += [v_unit(0, 0), v_unit(0, 1)]
                us += [qk_unit(0, wq_sb, qt_sb, 2),
                       qk_unit(0, wk_sb, kt_sb, 2)]
                us += [v_unit(0, 2), v_unit(0, 3)]
                us += [qk_unit(0, wq_sb, qt_sb, 3),
                       qk_unit(0, wk_sb, kt_sb, 3)]
                return us

            # ---- attention: flat (qi, g, kt) stream, SW pipelined ----
            inv8 = 1.0 / float(np.sqrt(DH))
            phase_units = {
                0: units_t0_rest() + units_t(1),
                1: units_t(2),
                2: units_t(3),
                3: units_o(0) + units_o(1) + units_o(2),
            }

            def do_pv(y_ps, g, kt, pt2, q_lo, nkt):
                for hh in range(2):
                    nc.tensor.matmul(
                        y_ps[hh][:, q_lo:TQ],
                        lhsT=v_sb[:, kt, 2 * g + hh, 0 : DH + 1],
                        rhs=pt2[:, hh, q_lo:TQ],
                        start=(kt == 0),
                        stop=(kt == nkt - 1),
                        skip_group_check=True,
                    )

            def do_epi_copies(qi, g, y_ps, q0, q1, tiles=None, use_act=False):
                # stage PSUM->SBUF: frees the y banks for the next g.  Both
                # heads stack on the partition axis of one [128,TQ] tile;
                # rowsum rows lane-shift 64->0 (plain copies support that;
                # the custom-DVE recip only works at base partition 0 on hw).
                # At the tail (use_act) the copies run on the then-idle ACT
                # engine so the recip chains behind them in the DVE queue
                # start sooner.
                if tiles is None:
                    yst = small.tile([P, TQ], F32, tag="yst", bufs=3,
                                     name=f"yst_{qi}_{g}")
                    rs = small.tile([1, 2, TQ], F32, tag="rs", bufs=3,
                                    name=f"rs_{qi}_{g}")
                else:
                    yst, rs = tiles

                def cp(dst, src):
                    if use_act:
                        nc.scalar.activation(dst, src, COPY)
                    else:
                        nc.vector.tensor_copy(dst, src)

                cp(yst[0:DH, q0:q1], y_ps[0][0:DH, q0:q1])
                cp(yst[DH : 2 * DH, q0:q1], y_ps[1][0:DH, q0:q1])
                for hh in range(2):
                    cp(rs[0:1, hh, q0:q1], y_ps[hh][DH : DH + 1, q0:q1])
                return yst, rs

            def do_epi_recip(qi, g, rs, q0, q1, cid=""):
                recipf = small.tile([1, 2, TQ], F32, tag="rf", bufs=3,
                                    name=f"rf_{qi}_{g}{cid}")
                nc.vector.reciprocal_approx_fast(
                    recipf[:, :, q0:q1], rs[:, :, q0:q1]
                )
                rcomp = small.tile([1, 2, TQ], BF16, tag="rc", bufs=3,
                                   name=f"rc_{qi}_{g}{cid}")
                nc.vector.tensor_copy(rcomp[:, :, q0:q1], recipf[:, :, q0:q1])
                return rcomp

            def do_epi_bc(qi, g, yst, rcomp, q0, q1):
                # two bf16 broadcast matmuls fill a [128,TQ] multiplier tile
                # (head h's 1/rowsum on partitions [64h:64h+64)); ONE DVE
                # multiply then normalizes both heads at once
                qsl = slice(qi * TQ + q0, qi * TQ + q1)
                bc = ppool.tile([P, TQ], F32, tag="mm",
                                name=f"bc_{qi}_{g}_{q0}")
                for hh in range(2):
                    nc.tensor.matmul(
                        bc[hh * DH : (hh + 1) * DH, q0:q1],
                        lhsT=ones_bf[:],
                        rhs=rcomp[0:1, hh, q0:q1],
                        start=True, stop=True,
                    )
                nc.vector.tensor_tensor(
                    yt_sb[:, g, qsl], yst[:, q0:q1], bc[:, q0:q1], MUL
                )

            def do_epi_norm(qi, g, yst, rs):
                do_epi_bc(qi, g, yst, do_epi_recip(qi, g, rs, 0, TQ), 0, TQ)

            # software pipeline state.  pend: PV (+ epilogue-copy) work
            # trailing the S/exp stream.  epi field: None, or
            # (q0, q1, defer) -- defer=True enqueues the deferred norm,
            # defer=False stashes the tiles for the custom tail flush.
            pend = []  # (y_ps, g, kt, pt2, q_lo, nkt, qi, epi)
            epiq = []
            fstate = {"filler": [], "issued": 0}
            tail = {}  # stashed (yst, rs) for the split (qi=3, g=3) epilogue

            def pop_pend():
                p = pend.pop(0)
                do_pv(*p[:6])
                epi = p[7]
                if epi is not None:
                    q0, q1, defer = epi
                    tiles = tail.get("t") if not defer else None
                    yst, rs = do_epi_copies(p[6], p[1], p[0], q0, q1, tiles,
                                            use_act=not defer)
                    if defer:
                        epiq.append((p[6], p[1], yst, rs))
                        if len(epiq) > 1:
                            do_epi_norm(*epiq.pop(0))
                    else:
                        tail["t"] = (yst, rs)

            def dispense(n=10**9):
                fl = fstate["filler"]
                done = 0
                while fstate["issued"] < len(fl) and done < n:
                    fl[fstate["issued"]]()
                    fstate["issued"] += 1
                    done += 1

            def flush_pend():
                while pend:
                    pop_pend()
                rcs = [(e, do_epi_recip(e[0], e[1], e[3], 0, TQ))
                       for e in epiq]
                epiq.clear()
                dispense()
                for e, rc in rcs:
                    do_epi_bc(e[0], e[1], e[2], rc, 0, TQ)

            def issue_s(qi, g, kt, y_ps, nkt):
                m = kt - 4 * qi
                q_lo = max(m, 0) * P if RESTRICT else 0
                qsl = slice(qi * TQ + q_lo, (qi + 1) * TQ)
                ksl = slice(kt * TK, (kt + 1) * TK)
                s2 = psS.tile([P, 2, TQ], F32, tag="att",
                              name=f"s_{qi}_{g}_{kt}")
                for hh in range(2):
                    hsl = slice(hh * DH, (hh + 1) * DH)
                    nc.tensor.matmul(
                        s2[:, hh, q_lo:TQ],
                        lhsT=kt_sb[hsl, g, ksl],
                        rhs=qt_sb[hsl, g, qsl],
                        start=True,
                        stop=True,
                    )
                return s2, q_lo, m

            def issue_exp(qi, g, kt, s2, q_lo, m):
                pt2 = ptp.tile([P, 2, TQ], BF16, tag="pt",
                               name=f"p_{qi}_{g}_{kt}")
                nc.scalar.activation(
                    pt2[:, :, q_lo:TQ], s2[:, :, q_lo:TQ],
                    EXP, scale=inv8,
                )
                if m >= 0:  # diagonal block: 0/1 triangle mask
                    if RESTRICT:
                        nc.vector.tensor_tensor(
                            pt2[:, :, q_lo : q_lo + P],
                            pt2[:, :, q_lo : q_lo + P],
                            tri_sb[:, 0:1, :].to_broadcast([P, 2, P]),
                            MUL,
                        )
                    else:
                        nc.vector.tensor_tensor(
                            pt2[:], pt2[:],
                            mask_sb[:, m : m + 1, :].to_broadcast(
                                [P, 2, TQ]
                            ),
                            MUL,
                        )
                return pt2

            def epi_for(qi, g, kt, nkt):
                if qi == NTT - 1 and g == NJT - 1:
                    if kt == nkt - 2:
                        return (0, QA, False)
                    if kt == nkt - 1:
                        return (QA, TQ, False)
                    return None
                return (0, TQ, True) if kt == nkt - 1 else None

            for qi in range(NTT):
                filler = phase_units[qi]
                total_f = len(filler)
                fstate["filler"] = filler
                fstate["issued"] = 0
                if not INTERLEAVE:
                    flush_pend()
                    dispense()
                nkt = 4 * qi + 4
                iters = nkt * NJT
                it = 0
                for g in range(NJT):
                    y_ps = [
                        psY.tile([DH + 1, TQ], F32, tag=f"y{hh}",
                                 name=f"y_{qi}_{g}_{hh}")
                        for hh in range(2)
                    ]
                    # kt stepped in pairs: the two S-pairs (and later the two
                    # PV-pairs) issue back-to-back so the PE pays half the
                    # weight-config switch overhead
                    for kt in range(0, nkt, 2):
                        sa = issue_s(qi, g, kt, y_ps, nkt)
                        sb = issue_s(qi, g, kt + 1, y_ps, nkt)
                        pa = issue_exp(qi, g, kt, *sa)
                        pb = issue_exp(qi, g, kt + 1, *sb)
                        it += 2
                        avail = total_f - (RESERVE if qi == NTT - 1 else 0)
                        while (fstate["issued"] * iters < total_f * it
                               and fstate["issued"] < avail):
                            filler[fstate["issued"]]()
                            fstate["issued"] += 1
                        pend.append((y_ps, g, kt, pa, sa[1], nkt, qi,
                                     epi_for(qi, g, kt, nkt)))
                        pend.append((y_ps, g, kt + 1, pb, sb[1], nkt, qi,
                                     epi_for(qi, g, kt + 1, nkt)))
                        # batch PV pops in fours (every other body) so the
                        # same-config PV matmuls chain with weight
                        # double-buffering; never start a new head-group's
                        # PVs mid-batch (its y banks need the previous
                        # group's drain slack)
                        if len(pend) >= 8:
                            popped = 0
                            while pend and popped < 4:
                                if popped >= 2 and pend[0][2] == 0:
                                    break
                                pop_pend()
                                popped += 1

            # ---- tail flush: drain the pipeline, normalize the last
            # group's A/B chunks, and run the qi=3 out-projection as
            # q-chunked chains with batched DMAs.  Reserved fillers are
            # interleaved WITH the recip chains (and drain via ACT) so the
            # PE never waits on the DVE queue; the whole tail stays dense,
            # which also keeps the PE p-state at max.
            t3A = slice(3 * TQ, 3 * TQ + QA)
            t3B = slice(3 * TQ + QA, 4 * TQ)
            fstate["drain"] = "act"
            while pend:
                pop_pend()
            dispense(1)
            rcs = [(e, do_epi_recip(e[0], e[1], e[3], 0, TQ)) for e in epiq]
            epiq.clear()
            ystT, rsT = tail["t"]
            rcA = do_epi_recip(3, 3, rsT, 0, QA, cid="A")
            dispense(1)
            for e, rc in rcs:
                do_epi_bc(e[0], e[1], e[2], rc, 0, TQ)
            do_epi_bc(3, 3, ystT, rcA, 0, QA)
            dispense()  # remaining reserve covers the mulA latency
            # A-chunk out-proj; B-norm DVE chain runs under the A chains
            oA = [o_tail_unit(ot, 0, QA, obA) for ot in range(NOT_)]
            oB = [o_tail_unit(ot, QA, TQ, obB) for ot in range(NOT_)]
            oA[0]()
            oA[1]()
            rcB = do_epi_recip(3, 3, rsT, QA, TQ, cid="B")
            oA[2]()
            oA[3]()
            nc.sync.dma_start(out=outPv[:, 0:4, t3A], in_=obA[:, 0:4, :])
            do_epi_bc(3, 3, ystT, rcB, QA, TQ)
            for ot in range(4, NOT_):
                oA[ot]()
            nc.sync.dma_start(out=outPv[:, 4:8, t3A], in_=obA[:, 4:8, :])
            for ot in range(NOT_):
                oB[ot]()
            nc.sync.dma_start(out=outPv[:, :, t3B], in_=obB[:])

    nc.compile()
    return nc


def make_in_maps(x, Wq, Wk, Wv, Wo):
    import ml_dtypes

    bf = ml_dtypes.bfloat16
    tri = np.triu(np.ones((P, P), dtype=np.float32)).astype(bf)
    k_ = np.arange(P)[:, None]
    q_ = np.arange(TQ)[None, :]
    mask4 = np.stack(
        [(q_ >= k_ + m * P) for m in range(4)]
    ).astype(np.float32).astype(bf)
    x = np.asarray(x, np.float32)
    Wq, Wk, Wv, Wo = (np.asarray(w, np.float32) for w in (Wq, Wk, Wv, Wo))
    def ilv(a2d, inner):
        # [D, F] -> dt-interleaved [P, F, NDT]: (p, f, n) = a2d[n*128+p, f]
        return np.ascontiguousarray(
            a2d.reshape(NDT, P, inner).transpose(1, 2, 0)
        ).astype(bf)

    in_maps = []
    for c in range(NCORES):
        b, hg = c // 2, c % 2
        sl = slice(hg * JJ, (hg + 1) * JJ)
        in_maps.append({
            "xS": ilv(x[b].T, T),
            "wqS": ilv(Wq[sl].T, JJ),
            "wkS": ilv(Wk[sl].T, JJ),
            "wvT": np.ascontiguousarray(Wv[sl].T).astype(bf),
            "woT": np.ascontiguousarray(Wo[:, sl].T).astype(bf),
            "tri": tri,
            "mask": mask4,
        })
    return in_maps


def gather_output(results):
    out = np.zeros((B, T, D), np.float32)
    for c in range(NCORES):
        out[c // 2] += results[c]["outT"].T.astype(np.float32)
    return out


def kernel(x, Wq, Wk, Wv, Wo):
    nc = build_program()
    in_maps = make_in_maps(x, Wq, Wk, Wv, Wo)
    res = run_bass_kernel_spmd(nc, in_maps, list(range(NCORES)))
    return gather_output(res.results)


if __name__ == "__main__":
    rng = np.random.default_rng(0)
    xs = [rng.standard_normal(s, dtype=np.float32) for s in
          [(B, T, D), (D, D), (D, D), (D, D), (D, D)]]
    out = kernel(*xs)
    print(out.shape, out.dtype)


# revision 22
# speedup vs baseline: 1.7582x; 1.7582x over previous
"""Causal self-attention on 8 Trainium2 NeuronCores.

Problem: x[4,2048,1024], Wq/Wk/Wv/Wo[1024,1024], H=16 heads, dh=64.
    q,k,v = x@W{q,k,v}.T ; per-head causal softmax(q k^T/8) v ; out = y@Wo.T

Sharding (hybrid data+tensor parallel over 8 cores):
  core c -> (batch b = c//2, head-group hg = c%2 of 8 heads = 512 dims).
  Each core computes a partial output out_c[b] = y_hg @ Wo[:, hg].T ; the
  host sums the two partials per batch (the Wo all-reduce done on host).

Per-core kernel: one software-pipelined instruction stream.
  The attention inner loop (S^T matmul -> exp on ACT -> PV matmul) is
  ACT-bound per iteration, while the QKV/output projections are pure PE
  work with idle ACT.  So projection matmul "filler units" are interleaved
  INTO the attention kt-loop so the PE never waits for exp:
    prologue   : DMA (small first chunks in consumption order), QKV t0
    attn(q0)   : filler = QKV(t1)      attn(q1): filler = QKV(t2)
    attn(q2)   : filler = QKV(t3)      attn(q3): filler = out-proj(q0..q2)
    epilogue   : out-proj(q3) split in two q-chunks (see below)
  kt is stepped in pairs and the PV-pairs trail the S-pairs by a few
  kt-steps so neither the exp latency nor the PSUM drains gate the PE.

  Causal restriction: for a diagonal k-tile (tile-local index m), only
  q >= 128*m can see it, so S/exp/PV operate on the q-slice [128m:512] of
  the q-tile, and the 0/1 triangle mask multiply only touches the single
  128-wide q-block on the diagonal itself.

  Softmax normalization per (head-pair g): V carries a ones column so PV
  also accumulates the row-sum (row 64 of y_ps).  After the last PV the
  y banks drain to a [128,TQ] SBUF tile (both heads stacked on the
  partition axis) and the rowsums lane-shift to partitions 0/1.  Then --
  deferred one head-group so it never delays the next group's masks in
  the DVE queue -- one batched reciprocal, a bf16 cast, ONE selector
  matmul (lhsT = E[2,128] with E[0,0:64]=1, E[1,64:128]=1) broadcasts
  1/rowsum_h0 across partitions 0:64 and 1/rowsum_h1 across 64:128 in a
  single PE op, and ONE DVE multiply writes the normalized bf16 yT.

  Tail: y(qi=3) for q in [0,384) is final after the kt=14 PV (kt=15 only
  touches q>=384), so the last group's epilogue splits in an A-chunk
  [0:384) and B-chunk [384:512).  The final out-projection runs as 8
  384-wide chains (A) + 8 128-wide chains (B) whose PSUM drains go to the
  then-idle ACT engine, results staged in two SBUF tiles and written with
  3 batched DMAs -- so the post-attention tail is short and dense instead
  of 8 full-width chains + 8 serial DMA triggers.

Precision: all matmul operands bf16 (PSUM accum fp32); softmax recip
f32, broadcast via bf16 (adds ~0.2% rms, budget is 2e-2). exp needs no
max-subtraction: S/8 ~ N(0,1), exp safe in fp32.  fp8 was evaluated and
rejected: e4m3 rms quantization error ~2.5% propagates ~1:1 through any
single matmul stage into the output and would blow the 2e-2 budget.

Measured on hw: 407us (naive) -> 265us (v1) -> this version.
"""

import sys

import numpy as np

sys.path.insert(0, "/opt/trn_rl_repo")

import concourse.bass as bass  # noqa: F401
from concourse import bacc
import concourse.mybir as mybir
import concourse.tile as tile
from concourse.bass_utils import run_bass_kernel_spmd

B, T, D, H, DH = 4, 2048, 1024, 16, 64
NCORES = 8
HPC = 8                 # heads per core
JJ = HPC * DH           # 512: per-core qkv head dims
P = 128
TQ = 512                # attention q tile (free dim of S^T matmul)
TK = 128                # attention k tile (partition dim of S^T)
NDT = D // P            # 8 d-tiles (contraction for stage 1)
NJT = JJ // P           # 4 j-tiles (head-pair tiles)
NTT = T // TQ           # 4 t-tiles of 512
NKT = T // TK           # 16 k-tiles of 128
NOT_ = D // P           # 8 output row tiles (stage 3)
VW = 66                 # V row width: 64 dh + 1 ones + 1 pad
QA = 384                # tail A-chunk width (final after kt=nkt-2)
F32 = mybir.dt.float32
BF16 = mybir.dt.bfloat16
MUL = mybir.AluOpType.mult
EXP = mybir.ActivationFunctionType.Exp
COPY = mybir.ActivationFunctionType.Copy
INTERLEAVE = True   # dispense filler units inside the attention kt loop
RESTRICT = True     # causal q-column restriction on diagonal k-tiles
RESERVE = 4         # fillers held back to cover the tail epilogue chains


def build_program():
    nc = bacc.Bacc()
    # NOTE: a dt-interleaved DRAM/SBUF layout (for bigger DMA descriptors)
    # was tried and reverted: strided PE ifmap reads run ~2.4x slower, which
    # costs far more than the startup DMA latency it saved.
    xT = nc.dram_tensor("xT", [D, T], BF16, kind="ExternalInput")
    wqT = nc.dram_tensor("wqT", [D, JJ], BF16, kind="ExternalInput")
    wkT = nc.dram_tensor("wkT", [D, JJ], BF16, kind="ExternalInput")
    wvT = nc.dram_tensor("wvT", [D, JJ], BF16, kind="ExternalInput")
    woT = nc.dram_tensor("woT", [JJ, D], BF16, kind="ExternalInput")
    trid = nc.dram_tensor("tri", [P, P], BF16, kind="ExternalInput")
    maskd = nc.dram_tensor("mask", [4, P, TQ], BF16, kind="ExternalInput")
    outT = nc.dram_tensor("outT", [D, T], BF16, kind="ExternalOutput")

    outv = outT.rearrange("(n p) t -> n p t", p=P)     # [8,128,2048]
    outPv = outT.rearrange("(n p) t -> p n t", p=P)    # [128,8,2048]

    with tile.TileContext(nc) as tc:
        with (
            tc.tile_pool(name="persist", bufs=1) as persist,
            tc.tile_pool(name="ppool", bufs=2, space="PSUM") as ppool,
            tc.tile_pool(name="psS", bufs=2, space="PSUM") as psS,
            tc.tile_pool(name="psY", bufs=1, space="PSUM") as psY,
            tc.tile_pool(name="ptp", bufs=10) as ptp,
            tc.tile_pool(name="small", bufs=2) as small,
        ):
            # ---- persistent SBUF tensors ----
            x_sb = persist.tile([P, NDT, T], BF16)        # x^T, d-tiled
            wq_sb = persist.tile([P, NDT, JJ], BF16)
            wk_sb = persist.tile([P, NDT, JJ], BF16)
            wv_sb = persist.tile([P, NDT, JJ], BF16)
            wo_sb = persist.tile([P, NJT, D], BF16)
            qt_sb = persist.tile([P, NJT, T], BF16)       # QT [j,t]
            kt_sb = persist.tile([P, NJT, T], BF16)       # KT [j,t]
            v_sb = persist.tile([P, NKT, HPC, VW], BF16)  # V'[t, kt, h, dh|1]
            yt_sb = persist.tile([P, NJT, T], BF16)       # yT [i,t] normalized
            tri_sb = persist.tile([P, 1, P], BF16)        # causal 0/1 triangle
            ones_bf = persist.tile([1, DH], BF16)         # bc lhsT
            obA = persist.tile([P, NOT_, QA], BF16)       # tail A out staging
            obB = persist.tile([P, NOT_, TQ - QA], BF16)  # tail B out staging

            # ones column of V' (strided memset across kt,h); bc ones row
            nc.any.memset(v_sb[:, :, :, DH : DH + 1], 1.0)
            nc.any.memset(ones_bf[:], 1.0)

            mask_sb = persist.tile([P, 4, TQ], BF16)

            # ---- DMAs: one strided transfer per tensor chunk (the
            # sync-queue trigger costs ~650ns each, so batch them), in
            # consumption order
            xPv = xT.rearrange("(n p) t -> p n t", p=P)     # [128,8,2048]
            wqP = wqT.rearrange("(n p) j -> p n j", p=P)    # [128,8,512]
            wkP = wkT.rearrange("(n p) j -> p n j", p=P)
            wvP = wvT.rearrange("(n p) j -> p n j", p=P)
            woP = woT.rearrange("(n p) o -> p n o", p=P)    # [128,4,1024]
            nc.sync.dma_start(out=wq_sb[:, :, 0:P], in_=wqP[:, :, 0:P])
            nc.sync.dma_start(out=x_sb[:, 0:2, 0:TQ], in_=xPv[:, 0:2, 0:TQ])
            nc.sync.dma_start(out=x_sb[:, 2:4, 0:TQ], in_=xPv[:, 2:4, 0:TQ])
            nc.sync.dma_start(out=x_sb[:, 4:6, 0:TQ], in_=xPv[:, 4:6, 0:TQ])
            nc.sync.dma_start(out=x_sb[:, 6:8, 0:TQ], in_=xPv[:, 6:8, 0:TQ])
            nc.sync.dma_start(out=wk_sb[:, :, 0:P], in_=wkP[:, :, 0:P])
            nc.sync.dma_start(out=tri_sb[:, 0, :], in_=trid[:, :])
            nc.sync.dma_start(out=wv_sb[:, 0:4, :], in_=wvP[:, 0:4, :])
            nc.sync.dma_start(out=wv_sb[:, 4:8, :], in_=wvP[:, 4:8, :])
            if not RESTRICT:
                for m in range(4):
                    nc.sync.dma_start(out=mask_sb[:, m, :], in_=maskd[m])
            nc.sync.dma_start(out=wq_sb[:, :, P:JJ], in_=wqP[:, :, P:JJ])
            nc.sync.dma_start(out=wk_sb[:, :, P:JJ], in_=wkP[:, :, P:JJ])
            nc.sync.dma_start(out=x_sb[:, :, TQ:T], in_=xPv[:, :, TQ:T])
            nc.sync.dma_start(out=wo_sb[:], in_=woP)

            # ---- filler units (each: one PSUM matmul chain + drain) ----
            def qk_unit(ti, w_sb, o_sb, jt):
                def run():
                    tsl = slice(ti * TQ, (ti + 1) * TQ)
                    jsl = slice(jt * P, (jt + 1) * P)
                    ps = ppool.tile([P, TQ], F32, tag="mm",
                                    name=f"qk_{ti}_{jt}")
                    for dt_ in range(NDT):
                        nc.tensor.matmul(
                            ps[:],
                            lhsT=w_sb[:, dt_, jsl],
                            rhs=x_sb[:, dt_, tsl],
                            start=(dt_ == 0),
                            stop=(dt_ == NDT - 1),
                        )
                    nc.vector.tensor_copy(o_sb[:, jt, tsl], ps[:])
                return run

            def v_unit(ti, tsub):
                def run():
                    kt_idx = ti * (TQ // P) + tsub
                    ssl = slice(ti * TQ + tsub * P, ti * TQ + (tsub + 1) * P)
                    ps = ppool.tile([P, JJ], F32, tag="mm",
                                    name=f"v_{kt_idx}")
                    for dt_ in range(NDT):
                        nc.tensor.matmul(
                            ps[:],
                            lhsT=x_sb[:, dt_, ssl],
                            rhs=wv_sb[:, dt_, :],
                            start=(dt_ == 0),
                            stop=(dt_ == NDT - 1),
                        )
                    nc.scalar.activation(
                        v_sb[:, kt_idx, :, 0:DH],
                        ps[:].rearrange("p (h i) -> p h i", h=HPC),
                        COPY,
                    )
                return run

            def o_unit(qi, ot):
                def run():
                    tsl = slice(qi * TQ, (qi + 1) * TQ)
                    osl = slice(ot * P, (ot + 1) * P)
                    ps = ppool.tile([P, TQ], F32, tag="mm",
                                    name=f"o_{qi}_{ot}")
                    for it in range(NJT):
                        nc.tensor.matmul(
                            ps[:],
                            lhsT=wo_sb[:, it, osl],
                            rhs=yt_sb[:, it, tsl],
                            start=(it == 0),
                            stop=(it == NJT - 1),
                        )
                    ob = small.tile([P, TQ], BF16, tag="ost", bufs=3,
                                    name=f"ob_{qi}_{ot}")
                    if fstate.get("drain") == "act":
                        # tail: ACT is done with exp; draining there keeps
                        # the PE's filler stream independent of the DVE
                        # queue (which is busy with the epilogue chains)
                        nc.scalar.activation(ob[:], ps[:], COPY)
                    else:
                        nc.vector.tensor_copy(ob[:], ps[:])
                    nc.sync.dma_start(out=outv[ot][:, tsl], in_=ob[:])
                return run

            # tail out-proj: q-chunked chains, ACT drains into staging tiles
            def o_tail_unit(ot, q0, q1, stage):
                def run():
                    tsl = slice(3 * TQ + q0, 3 * TQ + q1)
                    osl = slice(ot * P, (ot + 1) * P)
                    ps = ppool.tile([P, TQ], F32, tag="mm",
                                    name=f"oT_{ot}_{q0}")
                    for it in range(NJT):
                        nc.tensor.matmul(
                            ps[:, 0 : q1 - q0],
                            lhsT=wo_sb[:, it, osl],
                            rhs=yt_sb[:, it, tsl],
                            start=(it == 0),
                            stop=(it == NJT - 1),
                        )
                    nc.scalar.activation(
                        stage[:, ot, :], ps[:, 0 : q1 - q0], COPY
                    )
                return run

            def units_t(ti):
                return (
                    [qk_unit(ti, wq_sb, qt_sb, jt) for jt in range(NJT)]
                    + [qk_unit(ti, wk_sb, kt_sb, jt) for jt in range(NJT)]
                    + [v_unit(ti, ts) for ts in range(TQ // P)]
                )

            def units_o(qi):
                return [o_unit(qi, ot) for ot in range(NOT_)]

            # ---- prologue: only the two units attn(q0, g0) needs ----
            qk_unit(0, wq_sb, qt_sb, 0)()
            qk_unit(0, wk_sb, kt_sb, 0)()

            def units_t0_rest():
                # qk(g1) early for the g1 S-stream; V before the first pops
                us = [qk_unit(0, wq_sb, qt_sb, 1),
                      qk_unit(0, wk_sb, kt_sb, 1)]
                us += [v_unit(0, 0), v_unit(0, 1)]
                us += [qk_unit(0, wq_sb, qt_sb, 2),
                       qk_unit(0, wk_sb, kt_sb, 2)]
                us += [v_unit(0, 2), v_unit(0, 3)]
                us += [qk_unit(0, wq_sb, qt_sb, 3),
                       qk_unit(0, wk_sb, kt_sb, 3)]
                return us

            # ---- attention: flat (qi, g, kt) stream, SW pipelined ----
            inv8 = 1.0 / float(np.sqrt(DH))
            phase_units = {
                0: units_t0_rest() + units_t(1),
                1: units_t(2),
                2: units_t(3),
                3: units_o(0) + units_o(1) + units_o(2),
            }

            def do_pv(y_ps, g, kt, pt2, q_lo, nkt):
                for hh in range(2):
                    nc.tensor.matmul(
                        y_ps[hh][:, q_lo:TQ],
                        lhsT=v_sb[:, kt, 2 * g + hh, 0 : DH + 1],
                        rhs=pt2[:, hh, q_lo:TQ],
                        start=(kt == 0),
                        stop=(kt == nkt - 1),
                        skip_group_check=True,
                    )

            def do_epi_copies(qi, g, y_ps, q0, q1, tiles=None, use_act=False):
                # stage PSUM->SBUF: frees the y banks for the next g.  Both
                # heads stack on the partition axis of one [128,TQ] tile;
                # rowsum rows lane-shift 64->0 (plain copies support that;
                # the custom-DVE recip only works at base partition 0 on hw).
                # At the tail (use_act) the copies run on the then-idle ACT
                # engine so the recip chains behind them in the DVE queue
                # start sooner.
                if tiles is None:
                    yst = small.tile([P, TQ], F32, tag="yst", bufs=3,
                                     name=f"yst_{qi}_{g}")
                    rs = small.tile([1, 2, TQ], F32, tag="rs", bufs=3,
                                    name=f"rs_{qi}_{g}")
                else:
                    yst, rs = tiles

                def cp(dst, src):
                    if use_act:
                        nc.scalar.activation(dst, src, COPY)
                    else:
                        nc.vector.tensor_copy(dst, src)

                cp(yst[0:DH, q0:q1], y_ps[0][0:DH, q0:q1])
                cp(yst[DH : 2 * DH, q0:q1], y_ps[1][0:DH, q0:q1])
                for hh in range(2):
                    cp(rs[0:1, hh, q0:q1], y_ps[hh][DH : DH + 1, q0:q1])
                return yst, rs

            def do_epi_recip(qi, g, rs, q0, q1, cid=""):
                recipf = small.tile([1, 2, TQ], F32, tag="rf", bufs=3,
                                    name=f"rf_{qi}_{g}{cid}")
                nc.vector.reciprocal_approx_fast(
                    recipf[:, :, q0:q1], rs[:, :, q0:q1]
                )
                rcomp = small.tile([1, 2, TQ], BF16, tag="rc", bufs=3,
                                   name=f"rc_{qi}_{g}{cid}")
                nc.vector.tensor_copy(rcomp[:, :, q0:q1], recipf[:, :, q0:q1])
                return rcomp

            def do_epi_bc(qi, g, yst, rcomp, q0, q1):
                # two bf16 broadcast matmuls fill a [128,TQ] multiplier tile
                # (head h's 1/rowsum on partitions [64h:64h+64)); ONE DVE
                # multiply then normalizes both heads at once
                qsl = slice(qi * TQ + q0, qi * TQ + q1)
                bc = ppool.tile([P, TQ], F32, tag="mm",
                                name=f"bc_{qi}_{g}_{q0}")
                for hh in range(2):
                    nc.tensor.matmul(
                        bc[hh * DH : (hh + 1) * DH, q0:q1],
                        lhsT=ones_bf[:],
                        rhs=rcomp[0:1, hh, q0:q1],
                        start=True, stop=True,
                    )
                nc.vector.tensor_tensor(
                    yt_sb[:, g, qsl], yst[:, q0:q1], bc[:, q0:q1], MUL
                )

            def do_epi_norm(qi, g, yst, rs):
                do_epi_bc(qi, g, yst, do_epi_recip(qi, g, rs, 0, TQ), 0, TQ)

            # software pipeline state.  pend: PV (+ epilogue-copy) work
            # trailing the S/exp stream.  epi field: None, or
            # (q0, q1, defer) -- defer=True enqueues the deferred norm,
            # defer=False stashes the tiles for the custom tail flush.
            pend = []  # (y_ps, g, kt, pt2, q_lo, nkt, qi, epi)
            epiq = []
            fstate = {"filler": [], "issued": 0}
            tail = {}  # stashed (yst, rs) for the split (qi=3, g=3) epilogue

            def pop_pend():
                p = pend.pop(0)
                do_pv(*p[:6])
                epi = p[7]
                if epi is not None:
                    q0, q1, defer = epi
                    tiles = tail.get("t") if not defer else None
                    yst, rs = do_epi_copies(p[6], p[1], p[0], q0, q1, tiles,
                                            use_act=not defer)
                    if defer:
                        epiq.append((p[6], p[1], yst, rs))
                        if len(epiq) > 1:
                            do_epi_norm(*epiq.pop(0))
                    else:
                        tail["t"] = (yst, rs)

            def dispense(n=10**9):
                fl = fstate["filler"]
                done = 0
                while fstate["issued"] < len(fl) and done < n:
                    fl[fstate["issued"]]()
                    fstate["issued"] += 1
                    done += 1

            def flush_pend():
                while pend:
                    pop_pend()
                rcs = [(e, do_epi_recip(e[0], e[1], e[3], 0, TQ))
                       for e in epiq]
                epiq.clear()
                dispense()
                for e, rc in rcs:
                    do_epi_bc(e[0], e[1], e[2], rc, 0, TQ)

            def issue_s(qi, g, kt, y_ps, nkt):
                m = kt - 4 * qi
                q_lo = max(m, 0) * P if RESTRICT else 0
                qsl = slice(qi * TQ + q_lo, (qi + 1) * TQ)
                ksl = slice(kt * TK, (kt + 1) * TK)
                s2 = psS.tile([P, 2, TQ], F32, tag="att",
                              name=f"s_{qi}_{g}_{kt}")
                for hh in range(2):
                    hsl = slice(hh * DH, (hh + 1) * DH)
                    nc.tensor.matmul(
                        s2[:, hh, q_lo:TQ],
                        lhsT=kt_sb[hsl, g, ksl],
                        rhs=qt_sb[hsl, g, qsl],
                        start=True,
                        stop=True,
                    )
                return s2, q_lo, m

            def issue_exp(qi, g, kt, s2, q_lo, m):
                pt2 = ptp.tile([P, 2, TQ], BF16, tag="pt",
                               name=f"p_{qi}_{g}_{kt}")
                nc.scalar.activation(
                    pt2[:, :, q_lo:TQ], s2[:, :, q_lo:TQ],
                    EXP, scale=inv8,
                )
                if m >= 0:  # diagonal block: 0/1 triangle mask
                    if RESTRICT:
                        nc.vector.tensor_tensor(
                            pt2[:, :, q_lo : q_lo + P],
                            pt2[:, :, q_lo : q_lo + P],
                            tri_sb[:, 0:1, :].to_broadcast([P, 2, P]),
                            MUL,
                        )
                    else:
                        nc.vector.tensor_tensor(
                            pt2[:], pt2[:],
                            mask_sb[:, m : m + 1, :].to_broadcast(
                                [P, 2, TQ]
                            ),
                            MUL,
                        )
                return pt2

            def epi_for(qi, g, kt, nkt):
                if qi == NTT - 1 and g == NJT - 1:
                    if kt == nkt - 2:
                        return (0, QA, False)
                    if kt == nkt - 1:
                        return (QA, TQ, False)
                    return None
                return (0, TQ, True) if kt == nkt - 1 else None

            for qi in range(NTT):
                filler = phase_units[qi]
                total_f = len(filler)
                fstate["filler"] = filler
                fstate["issued"] = 0
                if not INTERLEAVE:
                    flush_pend()
                    dispense()
                nkt = 4 * qi + 4
                iters = nkt * NJT
                it = 0
                for g in range(NJT):
                    y_ps = [
                        psY.tile([DH + 1, TQ], F32, tag=f"y{hh}",
                                 name=f"y_{qi}_{g}_{hh}")
                        for hh in range(2)
                    ]
                    # kt stepped in pairs: the two S-pairs (and later the two
                    # PV-pairs) issue back-to-back so the PE pays half the
                    # weight-config switch overhead
                    for kt in range(0, nkt, 2):
                        sa = issue_s(qi, g, kt, y_ps, nkt)
                        sb = issue_s(qi, g, kt + 1, y_ps, nkt)
                        pa = issue_exp(qi, g, kt, *sa)
                        pb = issue_exp(qi, g, kt + 1, *sb)
                        it += 2
                        avail = total_f - (RESERVE if qi == NTT - 1 else 0)
                        while (fstate["issued"] * iters < total_f * it
                               and fstate["issued"] < avail):
                            filler[fstate["issued"]]()
                            fstate["issued"] += 1
                        pend.append((y_ps, g, kt, pa, sa[1], nkt, qi,
                                     epi_for(qi, g, kt, nkt)))
                        pend.append((y_ps, g, kt + 1, pb, sb[1], nkt, qi,
                                     epi_for(qi, g, kt + 1, nkt)))
                        # batch PV pops in fours (every other body) so the
                        # same-config PV matmuls chain with weight
                        # double-buffering; never start a new head-group's
                        # PVs mid-batch (its y banks need the previous
                        # group's drain slack)
                        if len(pend) >= 8:
                            popped = 0
                            while pend and popped < 4:
                                if popped >= 2 and pend[0][2] == 0:
                                    break
                                pop_pend()
                                popped += 1

            # ---- tail flush: drain the pipeline, normalize the last
            # group's A/B chunks, and run the qi=3 out-projection as
            # q-chunked chains with batched DMAs.  Reserved fillers are
            # interleaved WITH the recip chains (and drain via ACT) so the
            # PE never waits on the DVE queue; the whole tail stays dense,
            # which also keeps the PE p-state at max.
            t3A = slice(3 * TQ, 3 * TQ + QA)
            t3B = slice(3 * TQ + QA, 4 * TQ)
            fstate["drain"] = "act"
            while pend:
                pop_pend()
            dispense(1)
            rcs = [(e, do_epi_recip(e[0], e[1], e[3], 0, TQ)) for e in epiq]
            epiq.clear()
            ystT, rsT = tail["t"]
            rcA = do_epi_recip(3, 3, rsT, 0, QA, cid="A")
            dispense(1)
            for e, rc in rcs:
                do_epi_bc(e[0], e[1], e[2], rc, 0, TQ)
            do_epi_bc(3, 3, ystT, rcA, 0, QA)
            dispense()  # remaining reserve covers the mulA latency
            # A-chunk out-proj; B-norm DVE chain runs under the A chains
            oA = [o_tail_unit(ot, 0, QA, obA) for ot in range(NOT_)]
            oB = [o_tail_unit(ot, QA, TQ, obB) for ot in range(NOT_)]
            oA[0]()
            oA[1]()
            rcB = do_epi_recip(3, 3, rsT, QA, TQ, cid="B")
            oA[2]()
            oA[3]()
            nc.sync.dma_start(out=outPv[:, 0:4, t3A], in_=obA[:, 0:4, :])
            do_epi_bc(3, 3, ystT, rcB, QA, TQ)
            for ot in range(4, NOT_):
                oA[ot]()
            nc.sync.dma_start(out=outPv[:, 4:8, t3A], in_=obA[:, 4:8, :])
            for ot in range(NOT_):
                oB[ot]()
            nc.sync.dma_start(out=outPv[:, :, t3B], in_=obB[:])

    nc.compile()
    return nc


def make_in_maps(x, Wq, Wk, Wv, Wo):
    import ml_dtypes

    bf = ml_dtypes.bfloat16
    tri = np.triu(np.ones((P, P), dtype=np.float32)).astype(bf)
    k_ = np.arange(P)[:, None]
    q_ = np.arange(TQ)[None, :]
    mask4 = np.stack(
        [(q_ >= k_ + m * P) for m in range(4)]
    ).astype(np.float32).astype(bf)
    x = np.asarray(x, np.float32)
    Wq, Wk, Wv, Wo = (np.asarray(w, np.float32) for w in (Wq, Wk, Wv, Wo))
    in_maps = []
    for c in range(NCORES):
        b, hg = c // 2, c % 2
        sl = slice(hg * JJ, (hg + 1) * JJ)
        in_maps.append({
            "xT": np.ascontiguousarray(x[b].T).astype(bf),
            "wqT": np.ascontiguousarray(Wq[sl].T).astype(bf),
            "wkT": np.ascontiguousarray(Wk[sl].T).astype(bf),
            "wvT": np.ascontiguousarray(Wv[sl].T).astype(bf),
            "woT": np.ascontiguousarray(Wo[:, sl].T).astype(bf),
            "tri": tri,
            "mask": mask4,
        })
    return in_maps


def gather_output(results):
    out = np.zeros((B, T, D), np.float32)
    for c in range(NCORES):
        out[c // 2] += results[c]["outT"].T.astype(np.float32)
    return out


def kernel(x, Wq, Wk, Wv, Wo):
    nc = build_program()
    in_maps = make_in_maps(x, Wq, Wk, Wv, Wo)
    res = run_bass_kernel_spmd(nc, in_maps, list(range(NCORES)))
    return gather_output(res.results)


if __name__ == "__main__":
    rng = np.random.default_rng(0)
    xs = [rng.standard_normal(s, dtype=np.float32) for s in
          [(B, T, D), (D, D), (D, D), (D, D), (D, D)]]
    out = kernel(*xs)
    print(out.shape, out.dtype)


# revision 26
# speedup vs baseline: 1.7893x; 1.0177x over previous
"""Causal self-attention on 8 Trainium2 NeuronCores.

Problem: x[4,2048,1024], Wq/Wk/Wv/Wo[1024,1024], H=16 heads, dh=64.
    q,k,v = x@W{q,k,v}.T ; per-head causal softmax(q k^T/8) v ; out = y@Wo.T

Sharding (hybrid data+tensor parallel over 8 cores):
  core c -> (batch b = c//2, head-group hg = c%2 of 8 heads = 512 dims).
  Each core computes a partial output out_c[b] = y_hg @ Wo[:, hg].T ; the
  host sums the two partials per batch (the Wo all-reduce done on host).

Per-core kernel: one software-pipelined instruction stream.
  The attention inner loop (S^T matmul -> exp on ACT -> PV matmul) is
  ACT-bound per iteration, while the QKV/output projections are pure PE
  work with idle ACT.  So projection matmul "filler units" are interleaved
  INTO the attention kt-loop so the PE never waits for exp:
    prologue   : DMA (small first chunks in consumption order), QKV t0
    attn(q0)   : filler = QKV(t1)      attn(q1): filler = QKV(t2)
    attn(q2)   : filler = QKV(t3)      attn(q3): filler = out-proj(q0..q2)
    epilogue   : out-proj(q3) split in two q-chunks (see below)
  kt is stepped in pairs and the PV-pairs trail the S-pairs by a few
  kt-steps so neither the exp latency nor the PSUM drains gate the PE.

  Causal restriction: for a diagonal k-tile (tile-local index m), only
  q >= 128*m can see it, so S/exp/PV operate on the q-slice [128m:512] of
  the q-tile, and the 0/1 triangle mask multiply only touches the single
  128-wide q-block on the diagonal itself.

  Softmax normalization per (head-pair g): V carries a ones column so PV
  also accumulates the row-sum (row 64 of y_ps).  After the last PV the
  y banks drain to a [128,TQ] SBUF tile (both heads stacked on the
  partition axis) and the rowsums lane-shift to partitions 0/1.  Then --
  deferred one head-group so it never delays the next group's masks in
  the DVE queue -- one batched reciprocal, a bf16 cast, ONE selector
  matmul (lhsT = E[2,128] with E[0,0:64]=1, E[1,64:128]=1) broadcasts
  1/rowsum_h0 across partitions 0:64 and 1/rowsum_h1 across 64:128 in a
  single PE op, and ONE DVE multiply writes the normalized bf16 yT.

  Tail: y(qi=3) for q in [0,384) is final after the kt=14 PV (kt=15 only
  touches q>=384), so the last group's epilogue splits in an A-chunk
  [0:384) and B-chunk [384:512).  The final out-projection runs as 8
  384-wide chains (A) + 8 128-wide chains (B) whose PSUM drains go to the
  then-idle ACT engine, results staged in two SBUF tiles and written with
  3 batched DMAs -- so the post-attention tail is short and dense instead
  of 8 full-width chains + 8 serial DMA triggers.

Precision: all matmul operands bf16 (PSUM accum fp32); softmax recip
f32, broadcast via bf16 (adds ~0.2% rms, budget is 2e-2). exp needs no
max-subtraction: S/8 ~ N(0,1), exp safe in fp32.  fp8 was evaluated and
rejected: e4m3 rms quantization error ~2.5% propagates ~1:1 through any
single matmul stage into the output and would blow the 2e-2 budget.

Measured on hw: 407us (naive) -> 265us (v1) -> this version.
"""

import sys

import numpy as np

sys.path.insert(0, "/opt/trn_rl_repo")

import concourse.bass as bass  # noqa: F401
from concourse import bacc
import concourse.mybir as mybir
import concourse.tile as tile
from concourse.bass_utils import run_bass_kernel_spmd

B, T, D, H, DH = 4, 2048, 1024, 16, 64
NCORES = 8
HPC = 8                 # heads per core
JJ = HPC * DH           # 512: per-core qkv head dims
P = 128
TQ = 512                # attention q tile (free dim of S^T matmul)
TK = 128                # attention k tile (partition dim of S^T)
NDT = D // P            # 8 d-tiles (contraction for stage 1)
NJT = JJ // P           # 4 j-tiles (head-pair tiles)
NTT = T // TQ           # 4 t-tiles of 512
NKT = T // TK           # 16 k-tiles of 128
NOT_ = D // P           # 8 output row tiles (stage 3)
VW = 66                 # V row width: 64 dh + 1 ones + 1 pad
QA = 384                # tail A-chunk width (final after kt=nkt-2)
F32 = mybir.dt.float32
BF16 = mybir.dt.bfloat16
MUL = mybir.AluOpType.mult
EXP = mybir.ActivationFunctionType.Exp
COPY = mybir.ActivationFunctionType.Copy
INTERLEAVE = True   # dispense filler units inside the attention kt loop
RESTRICT = True     # causal q-column restriction on diagonal k-tiles
RESERVE = 4         # fillers held back to cover the tail epilogue chains


def build_program():
    nc = bacc.Bacc()
    # NOTE: a dt-interleaved DRAM/SBUF layout (for bigger DMA descriptors)
    # was tried and reverted: strided PE ifmap reads run ~2.4x slower, which
    # costs far more than the startup DMA latency it saved.
    xT = nc.dram_tensor("xT", [D, T], BF16, kind="ExternalInput")
    wqT = nc.dram_tensor("wqT", [D, JJ], BF16, kind="ExternalInput")
    wkT = nc.dram_tensor("wkT", [D, JJ], BF16, kind="ExternalInput")
    wvT = nc.dram_tensor("wvT", [D, JJ], BF16, kind="ExternalInput")
    woT = nc.dram_tensor("woT", [JJ, D], BF16, kind="ExternalInput")
    trid = nc.dram_tensor("tri", [P, P], BF16, kind="ExternalInput")
    maskd = nc.dram_tensor("mask", [4, P, TQ], BF16, kind="ExternalInput")
    outT = nc.dram_tensor("outT", [D, T], BF16, kind="ExternalOutput")

    outv = outT.rearrange("(n p) t -> n p t", p=P)     # [8,128,2048]
    outPv = outT.rearrange("(n p) t -> p n t", p=P)    # [128,8,2048]

    with tile.TileContext(nc) as tc:
        with (
            tc.tile_pool(name="persist", bufs=1) as persist,
            tc.tile_pool(name="ppool", bufs=2, space="PSUM") as ppool,
            tc.tile_pool(name="psS", bufs=2, space="PSUM") as psS,
            tc.tile_pool(name="psY", bufs=1, space="PSUM") as psY,
            tc.tile_pool(name="ptp", bufs=10) as ptp,
            tc.tile_pool(name="small", bufs=2) as small,
        ):
            # ---- persistent SBUF tensors ----
            x_sb = persist.tile([P, NDT, T], BF16)        # x^T, d-tiled
            wq_sb = persist.tile([P, NDT, JJ], BF16)
            wk_sb = persist.tile([P, NDT, JJ], BF16)
            wv_sb = persist.tile([P, NDT, JJ], BF16)
            wo_sb = persist.tile([P, NJT, D], BF16)
            qt_sb = persist.tile([P, NJT, T], BF16)       # QT [j,t]
            kt_sb = persist.tile([P, NJT, T], BF16)       # KT [j,t]
            v_sb = persist.tile([P, NKT, HPC, VW], BF16)  # V'[t, kt, h, dh|1]
            yt_sb = persist.tile([P, NJT, T], BF16)       # yT [i,t] normalized
            tri_sb = persist.tile([P, 1, P], BF16)        # causal 0/1 triangle
            ones_bf = persist.tile([1, DH], BF16)         # bc lhsT
            obA = persist.tile([P, NOT_, QA], BF16)       # tail A out staging
            obB = persist.tile([P, NOT_, TQ - QA], BF16)  # tail B out staging

            # ones column of V' (strided memset across kt,h); bc ones row
            nc.any.memset(v_sb[:, :, :, DH : DH + 1], 1.0)
            nc.any.memset(ones_bf[:], 1.0)

            mask_sb = persist.tile([P, 4, TQ], BF16)

            # ---- DMAs: one strided transfer per tensor chunk (the
            # sync-queue trigger costs ~650ns each, so batch them), in
            # consumption order
            xPv = xT.rearrange("(n p) t -> p n t", p=P)     # [128,8,2048]
            wqP = wqT.rearrange("(n p) j -> p n j", p=P)    # [128,8,512]
            wkP = wkT.rearrange("(n p) j -> p n j", p=P)
            wvP = wvT.rearrange("(n p) j -> p n j", p=P)
            woP = woT.rearrange("(n p) o -> p n o", p=P)    # [128,4,1024]
            nc.sync.dma_start(out=wq_sb[:, :, 0:P], in_=wqP[:, :, 0:P])
            nc.sync.dma_start(out=x_sb[:, 0:2, 0:TQ], in_=xPv[:, 0:2, 0:TQ])
            nc.sync.dma_start(out=x_sb[:, 2:4, 0:TQ], in_=xPv[:, 2:4, 0:TQ])
            nc.sync.dma_start(out=x_sb[:, 4:6, 0:TQ], in_=xPv[:, 4:6, 0:TQ])
            nc.sync.dma_start(out=x_sb[:, 6:8, 0:TQ], in_=xPv[:, 6:8, 0:TQ])
            nc.sync.dma_start(out=wk_sb[:, :, 0:P], in_=wkP[:, :, 0:P])
            nc.sync.dma_start(out=tri_sb[:, 0, :], in_=trid[:, :])
            nc.sync.dma_start(out=wv_sb[:, 0:4, :], in_=wvP[:, 0:4, :])
            nc.sync.dma_start(out=wv_sb[:, 4:8, :], in_=wvP[:, 4:8, :])
            if not RESTRICT:
                for m in range(4):
                    nc.sync.dma_start(out=mask_sb[:, m, :], in_=maskd[m])
            nc.sync.dma_start(out=wq_sb[:, :, P:JJ], in_=wqP[:, :, P:JJ])
            nc.sync.dma_start(out=wk_sb[:, :, P:JJ], in_=wkP[:, :, P:JJ])
            nc.sync.dma_start(out=x_sb[:, :, TQ:T], in_=xPv[:, :, TQ:T])
            nc.sync.dma_start(out=wo_sb[:], in_=woP)

            # ---- filler units (each: one PSUM matmul chain + drain) ----
            def qk_unit(ti, w_sb, o_sb, jt):
                def run():
                    tsl = slice(ti * TQ, (ti + 1) * TQ)
                    jsl = slice(jt * P, (jt + 1) * P)
                    ps = ppool.tile([P, TQ], F32, tag="mm",
                                    name=f"qk_{ti}_{jt}")
                    for dt_ in range(NDT):
                        nc.tensor.matmul(
                            ps[:],
                            lhsT=w_sb[:, dt_, jsl],
                            rhs=x_sb[:, dt_, tsl],
                            start=(dt_ == 0),
                            stop=(dt_ == NDT - 1),
                        )
                    nc.vector.tensor_copy(o_sb[:, jt, tsl], ps[:])
                return run

            def v_unit(ti, tsub):
                def run():
                    kt_idx = ti * (TQ // P) + tsub
                    ssl = slice(ti * TQ + tsub * P, ti * TQ + (tsub + 1) * P)
                    ps = ppool.tile([P, JJ], F32, tag="mm",
                                    name=f"v_{kt_idx}")
                    for dt_ in range(NDT):
                        nc.tensor.matmul(
                            ps[:],
                            lhsT=x_sb[:, dt_, ssl],
                            rhs=wv_sb[:, dt_, :],
                            start=(dt_ == 0),
                            stop=(dt_ == NDT - 1),
                        )
                    nc.scalar.activation(
                        v_sb[:, kt_idx, :, 0:DH],
                        ps[:].rearrange("p (h i) -> p h i", h=HPC),
                        COPY,
                    )
                return run

            def o_unit(qi, ot):
                def run():
                    tsl = slice(qi * TQ, (qi + 1) * TQ)
                    osl = slice(ot * P, (ot + 1) * P)
                    ps = ppool.tile([P, TQ], F32, tag="mm",
                                    name=f"o_{qi}_{ot}")
                    for it in range(NJT):
                        nc.tensor.matmul(
                            ps[:],
                            lhsT=wo_sb[:, it, osl],
                            rhs=yt_sb[:, it, tsl],
                            start=(it == 0),
                            stop=(it == NJT - 1),
                        )
                    ob = small.tile([P, TQ], BF16, tag="ost", bufs=3,
                                    name=f"ob_{qi}_{ot}")
                    if fstate.get("drain") == "act":
                        # tail: ACT is done with exp; draining there keeps
                        # the PE's filler stream independent of the DVE
                        # queue (which is busy with the epilogue chains)
                        nc.scalar.activation(ob[:], ps[:], COPY)
                    else:
                        nc.vector.tensor_copy(ob[:], ps[:])
                    nc.sync.dma_start(out=outv[ot][:, tsl], in_=ob[:])
                return run

            # tail out-proj: q-chunked chains, ACT drains into staging tiles
            def o_tail_unit(ot, q0, q1, stage):
                def run():
                    tsl = slice(3 * TQ + q0, 3 * TQ + q1)
                    osl = slice(ot * P, (ot + 1) * P)
                    ps = ppool.tile([P, TQ], F32, tag="mm",
                                    name=f"oT_{ot}_{q0}")
                    for it in range(NJT):
                        nc.tensor.matmul(
                            ps[:, 0 : q1 - q0],
                            lhsT=wo_sb[:, it, osl],
                            rhs=yt_sb[:, it, tsl],
                            start=(it == 0),
                            stop=(it == NJT - 1),
                        )
                    nc.scalar.activation(
                        stage[:, ot, :], ps[:, 0 : q1 - q0], COPY
                    )
                return run

            def units_t(ti):
                return (
                    [qk_unit(ti, wq_sb, qt_sb, jt) for jt in range(NJT)]
                    + [qk_unit(ti, wk_sb, kt_sb, jt) for jt in range(NJT)]
                    + [v_unit(ti, ts) for ts in range(TQ // P)]
                )

            def units_o(qi):
                return [o_unit(qi, ot) for ot in range(NOT_)]

            # ---- prologue: only the two units attn(q0, g0) needs ----
            qk_unit(0, wq_sb, qt_sb, 0)()
            qk_unit(0, wk_sb, kt_sb, 0)()

            def units_t0_rest():
                # ordered so V(kt) and QK(jt=g) land before their consumers
                # (and AFTER their weight DMAs: the wq/wk j>128 bulk lands
                # ~20us in, so the g1 units can't be first)
                us = [v_unit(0, 0), v_unit(0, 1)]
                us += [qk_unit(0, wq_sb, qt_sb, 1),
                       qk_unit(0, wk_sb, kt_sb, 1)]
                us += [v_unit(0, 2), v_unit(0, 3)]
                us += [qk_unit(0, wq_sb, qt_sb, 2),
                       qk_unit(0, wk_sb, kt_sb, 2),
                       qk_unit(0, wq_sb, qt_sb, 3),
                       qk_unit(0, wk_sb, kt_sb, 3)]
                return us

            # ---- attention: flat (qi, g, kt) stream, SW pipelined ----
            inv8 = 1.0 / float(np.sqrt(DH))
            phase_units = {
                0: units_t0_rest() + units_t(1),
                1: units_t(2),
                2: units_t(3),
                3: units_o(0) + units_o(1) + units_o(2),
            }

            def do_pv(y_ps, g, kt, pt2, q_lo, nkt):
                for hh in range(2):
                    nc.tensor.matmul(
                        y_ps[hh][:, q_lo:TQ],
                        lhsT=v_sb[:, kt, 2 * g + hh, 0 : DH + 1],
                        rhs=pt2[:, hh, q_lo:TQ],
                        start=(kt == 0),
                        stop=(kt == nkt - 1),
                        skip_group_check=True,
                    )

            def do_epi_copies(qi, g, y_ps, q0, q1, tiles=None, use_act=False):
                # stage PSUM->SBUF: frees the y banks for the next g.  Both
                # heads stack on the partition axis of one [128,TQ] tile;
                # rowsum rows lane-shift 64->0 (plain copies support that;
                # the custom-DVE recip only works at base partition 0 on hw).
                # Rowsums copy FIRST (the recip chain only needs them); at
                # the tail (use_act) the big yst copies go to the then-idle
                # ACT engine so they run concurrently with the DVE recips.
                if tiles is None:
                    yst = small.tile([P, TQ], F32, tag="yst", bufs=3,
                                     name=f"yst_{qi}_{g}")
                    rs = small.tile([1, 2, TQ], F32, tag="rs", bufs=3,
                                    name=f"rs_{qi}_{g}")
                else:
                    yst, rs = tiles
                for hh in range(2):
                    nc.vector.tensor_copy(
                        rs[0:1, hh, q0:q1], y_ps[hh][DH : DH + 1, q0:q1]
                    )
                eng = nc.scalar if use_act else nc.vector
                if use_act:
                    eng.activation(yst[0:DH, q0:q1], y_ps[0][0:DH, q0:q1],
                                   COPY)
                    eng.activation(yst[DH : 2 * DH, q0:q1],
                                   y_ps[1][0:DH, q0:q1], COPY)
                else:
                    eng.tensor_copy(yst[0:DH, q0:q1], y_ps[0][0:DH, q0:q1])
                    eng.tensor_copy(
                        yst[DH : 2 * DH, q0:q1], y_ps[1][0:DH, q0:q1]
                    )
                return yst, rs

            def do_epi_recip(qi, g, rs, q0, q1, cid=""):
                recipf = small.tile([1, 2, TQ], F32, tag="rf", bufs=3,
                                    name=f"rf_{qi}_{g}{cid}")
                nc.vector.reciprocal_approx_fast(
                    recipf[:, :, q0:q1], rs[:, :, q0:q1]
                )
                rcomp = small.tile([1, 2, TQ], BF16, tag="rc", bufs=3,
                                   name=f"rc_{qi}_{g}{cid}")
                nc.vector.tensor_copy(rcomp[:, :, q0:q1], recipf[:, :, q0:q1])
                return rcomp

            def do_epi_bc(qi, g, yst, rcomp, q0, q1):
                # two bf16 broadcast matmuls fill a [128,TQ] multiplier tile
                # (head h's 1/rowsum on partitions [64h:64h+64)); ONE DVE
                # multiply then normalizes both heads at once
                qsl = slice(qi * TQ + q0, qi * TQ + q1)
                bc = ppool.tile([P, TQ], F32, tag="mm",
                                name=f"bc_{qi}_{g}_{q0}")
                for hh in range(2):
                    nc.tensor.matmul(
                        bc[hh * DH : (hh + 1) * DH, q0:q1],
                        lhsT=ones_bf[:],
                        rhs=rcomp[0:1, hh, q0:q1],
                        start=True, stop=True,
                    )
                nc.vector.tensor_tensor(
                    yt_sb[:, g, qsl], yst[:, q0:q1], bc[:, q0:q1], MUL
                )

            def do_epi_norm(qi, g, yst, rs):
                do_epi_bc(qi, g, yst, do_epi_recip(qi, g, rs, 0, TQ), 0, TQ)

            # software pipeline state.  pend: PV (+ epilogue-copy) work
            # trailing the S/exp stream.  epi field: None, or
            # (q0, q1, defer) -- defer=True enqueues the deferred norm,
            # defer=False stashes the tiles for the custom tail flush.
            pend = []  # (y_ps, g, kt, pt2, q_lo, nkt, qi, epi)
            epiq = []
            fstate = {"filler": [], "issued": 0}
            tail = {}  # stashed (yst, rs) for the split (qi=3, g=3) epilogue

            def pop_pend():
                p = pend.pop(0)
                do_pv(*p[:6])
                epi = p[7]
                if epi is not None:
                    q0, q1, defer = epi
                    if defer:
                        yst, rs = do_epi_copies(p[6], p[1], p[0], q0, q1)
                        epiq.append((p[6], p[1], yst, rs))
                        if len(epiq) > 1:
                            do_epi_norm(*epiq.pop(0))
                    else:
                        # tail chunk: copies are deferred to the flush so
                        # they never impose a write-after-read wait on the
                        # PVs still accumulating into the same PSUM tile
                        tail.setdefault("chunks", []).append(
                            (p[6], p[1], p[0], q0, q1)
                        )

            def dispense(n=10**9):
                fl = fstate["filler"]
                done = 0
                while fstate["issued"] < len(fl) and done < n:
                    fl[fstate["issued"]]()
                    fstate["issued"] += 1
                    done += 1

            def flush_pend():
                while pend:
                    pop_pend()
                rcs = [(e, do_epi_recip(e[0], e[1], e[3], 0, TQ))
                       for e in epiq]
                epiq.clear()
                dispense()
                for e, rc in rcs:
                    do_epi_bc(e[0], e[1], e[2], rc, 0, TQ)

            def issue_s(qi, g, kt, y_ps, nkt):
                m = kt - 4 * qi
                q_lo = max(m, 0) * P if RESTRICT else 0
                qsl = slice(qi * TQ + q_lo, (qi + 1) * TQ)
                ksl = slice(kt * TK, (kt + 1) * TK)
                s2 = psS.tile([P, 2, TQ], F32, tag="att",
                              name=f"s_{qi}_{g}_{kt}")
                for hh in range(2):
                    hsl = slice(hh * DH, (hh + 1) * DH)
                    nc.tensor.matmul(
                        s2[:, hh, q_lo:TQ],
                        lhsT=kt_sb[hsl, g, ksl],
                        rhs=qt_sb[hsl, g, qsl],
                        start=True,
                        stop=True,
                    )
                return s2, q_lo, m

            def issue_exp(qi, g, kt, s2, q_lo, m):
                pt2 = ptp.tile([P, 2, TQ], BF16, tag="pt",
                               name=f"p_{qi}_{g}_{kt}")
                nc.scalar.activation(
                    pt2[:, :, q_lo:TQ], s2[:, :, q_lo:TQ],
                    EXP, scale=inv8,
                )
                if m >= 0:  # diagonal block: 0/1 triangle mask
                    if RESTRICT:
                        nc.vector.tensor_tensor(
                            pt2[:, :, q_lo : q_lo + P],
                            pt2[:, :, q_lo : q_lo + P],
                            tri_sb[:, 0:1, :].to_broadcast([P, 2, P]),
                            MUL,
                        )
                    else:
                        nc.vector.tensor_tensor(
                            pt2[:], pt2[:],
                            mask_sb[:, m : m + 1, :].to_broadcast(
                                [P, 2, TQ]
                            ),
                            MUL,
                        )
                return pt2

            def epi_for(qi, g, kt, nkt):
                if qi == NTT - 1 and g == NJT - 1:
                    if kt == nkt - 2:
                        return (0, QA, False)
                    if kt == nkt - 1:
                        return (QA, TQ, False)
                    return None
                return (0, TQ, True) if kt == nkt - 1 else None

            for qi in range(NTT):
                filler = phase_units[qi]
                total_f = len(filler)
                fstate["filler"] = filler
                fstate["issued"] = 0
                if not INTERLEAVE:
                    flush_pend()
                    dispense()
                nkt = 4 * qi + 4
                iters = nkt * NJT
                it = 0
                for g in range(NJT):
                    y_ps = [
                        psY.tile([DH + 1, TQ], F32, tag=f"y{hh}",
                                 name=f"y_{qi}_{g}_{hh}")
                        for hh in range(2)
                    ]
                    # kt stepped in pairs: the two S-pairs (and later the two
                    # PV-pairs) issue back-to-back so the PE pays half the
                    # weight-config switch overhead
                    for kt in range(0, nkt, 2):
                        sa = issue_s(qi, g, kt, y_ps, nkt)
                        sb = issue_s(qi, g, kt + 1, y_ps, nkt)
                        pa = issue_exp(qi, g, kt, *sa)
                        pb = issue_exp(qi, g, kt + 1, *sb)
                        it += 2
                        avail = total_f - (RESERVE if qi == NTT - 1 else 0)
                        while (fstate["issued"] * iters < total_f * it
                               and fstate["issued"] < avail):
                            filler[fstate["issued"]]()
                            fstate["issued"] += 1
                        pend.append((y_ps, g, kt, pa, sa[1], nkt, qi,
                                     epi_for(qi, g, kt, nkt)))
                        pend.append((y_ps, g, kt + 1, pb, sb[1], nkt, qi,
                                     epi_for(qi, g, kt + 1, nkt)))
                        # batch PV pops in fours (every other body) so the
                        # same-config PV matmuls chain with weight
                        # double-buffering; never start a new head-group's
                        # PVs mid-batch (its y banks need the previous
                        # group's drain slack)
                        if len(pend) >= 8:
                            popped = 0
                            while pend and popped < 4:
                                if popped >= 2 and pend[0][2] == 0:
                                    break
                                pop_pend()
                                popped += 1

            # ---- tail flush: drain the pipeline, normalize the last
            # group's A/B chunks, and run the qi=3 out-projection as
            # q-chunked chains with batched DMAs.  Reserved fillers are
            # interleaved WITH the recip chains (and drain via ACT) so the
            # PE never waits on the DVE queue; the whole tail stays dense,
            # which also keeps the PE p-state at max.
            t3A = slice(3 * TQ, 3 * TQ + QA)
            t3B = slice(3 * TQ + QA, 4 * TQ)
            fstate["drain"] = "act"
            while pend:
                pop_pend()
            # all PVs are issued; now stage the tail chunks (rowsums on DVE
            # first -- the recips gate on them -- yst bulk on idle ACT)
            tiles = None
            for (cqi, cg, cy_ps, q0, q1) in tail["chunks"]:
                tiles = do_epi_copies(cqi, cg, cy_ps, q0, q1, tiles,
                                      use_act=True)
            ystT, rsT = tiles
            dispense(1)
            rcs = [(e, do_epi_recip(e[0], e[1], e[3], 0, TQ)) for e in epiq]
            epiq.clear()
            rcA = do_epi_recip(3, 3, rsT, 0, QA, cid="A")
            dispense(1)
            for e, rc in rcs:
                do_epi_bc(e[0], e[1], e[2], rc, 0, TQ)
            do_epi_bc(3, 3, ystT, rcA, 0, QA)
            dispense()  # remaining reserve covers the mulA latency
            # A-chunk out-proj; B-norm DVE chain runs under the A chains
            oA = [o_tail_unit(ot, 0, QA, obA) for ot in range(NOT_)]
            oB = [o_tail_unit(ot, QA, TQ, obB) for ot in range(NOT_)]
            oA[0]()
            oA[1]()
            rcB = do_epi_recip(3, 3, rsT, QA, TQ, cid="B")
            oA[2]()
            oA[3]()
            nc.sync.dma_start(out=outPv[:, 0:4, t3A], in_=obA[:, 0:4, :])
            do_epi_bc(3, 3, ystT, rcB, QA, TQ)
            for ot in range(4, NOT_):
                oA[ot]()
            nc.sync.dma_start(out=outPv[:, 4:8, t3A], in_=obA[:, 4:8, :])
            for ot in range(NOT_):
                oB[ot]()
            nc.sync.dma_start(out=outPv[:, :, t3B], in_=obB[:])

    nc.compile()
    return nc


def make_in_maps(x, Wq, Wk, Wv, Wo):
    import ml_dtypes

    bf = ml_dtypes.bfloat16
    tri = np.triu(np.ones((P, P), dtype=np.float32)).astype(bf)
    k_ = np.arange(P)[:, None]
    q_ = np.arange(TQ)[None, :]
    mask4 = np.stack(
        [(q_ >= k_ + m * P) for m in range(4)]
    ).astype(np.float32).astype(bf)
    x = np.asarray(x, np.float32)
    Wq, Wk, Wv, Wo = (np.asarray(w, np.float32) for w in (Wq, Wk, Wv, Wo))
    in_maps = []
    for c in range(NCORES):
        b, hg = c // 2, c % 2
        sl = slice(hg * JJ, (hg + 1) * JJ)
        in_maps.append({
            "xT": np.ascontiguousarray(x[b].T).astype(bf),
            "wqT": np.ascontiguousarray(Wq[sl].T).astype(bf),
            "wkT": np.ascontiguousarray(Wk[sl].T).astype(bf),
            "wvT": np.ascontiguousarray(Wv[sl].T).astype(bf),
            "woT": np.ascontiguousarray(Wo[:, sl].T).astype(bf),
            "tri": tri,
            "mask": mask4,
        })
    return in_maps


def gather_output(results):
    out = np.zeros((B, T, D), np.float32)
    for c in range(NCORES):
        out[c // 2] += results[c]["outT"].T.astype(np.float32)
    return out


def kernel(x, Wq, Wk, Wv, Wo):
    nc = build_program()
    in_maps = make_in_maps(x, Wq, Wk, Wv, Wo)
    res = run_bass_kernel_spmd(nc, in_maps, list(range(NCORES)))
    return gather_output(res.results)


if __name__ == "__main__":
    rng = np.random.default_rng(0)
    xs = [rng.standard_normal(s, dtype=np.float32) for s in
          [(B, T, D), (D, D), (D, D), (D, D), (D, D)]]
    out = kernel(*xs)
    print(out.shape, out.dtype)
